# revision 9
# baseline (speedup 1.0000x reference)
"""GCN message-passing kernel for Trainium2 (8 NeuronCores, batch-parallel).

Model (see problem reference): two GCN layers over a fixed random graph
(N=1024 nodes, E=8192 directed edges, topology shared by all B=256
samples), LeakyReLU activations, global mean pool, Linear(64,128)+LeakyReLU.

Strategy (v2)
-------------
Because the topology is identical across the batch, GCN aggregation is a
fixed linear operator A_hat.  The baseline ran the layer-2 aggregation
densely over all 64 hidden features (131k PE rows/core).  v2 exploits the
low-rank structure of layer 1: h1[n] = LeakyReLU(<m_n, w_d>) is a function
of the 3-vector m_n = (A h0)[n] only.  Writing LeakyReLU(t) =
c1*t + c2*|t|, the batch-shared ridge decomposition

    |<m, w_d>|  ~=  sum_j C[j,d] * |<m, v_j>| ,   r=16 basis cones v_j

(fit on the host from W1 alone by least squares over isotropic m) turns
the layer-2 aggregation into:

    A h1 W2  ~=  c1 * (A^2 h0) (W1 W2)  +  c2 * (A |A h0 V|) (C W2)

so only r=16+4 features per sample are aggregated instead of 64, and A^2
is folded on the host.  End-to-end max error of the approximation +
bf16 operand rounding is ~6e-3 against the f64 reference (tolerance 2e-2).

All tensors are bf16 (1 PE row/cycle, same as fp32r, but half the DMA and
no fp32r staging-copy requirement); PSUM accumulation stays f32.  Inputs
are pre-transposed on the host so every DMA moves >=2KB contiguous lines.

Per core (32 samples), partitions x free:
  h0     [128 node, 8 kt, 32 b, 4 f]   (f=3 slot zero-padded)
  step1  M~_T  [128 (b,4f), 1024 n]  = sum_k h0[k].T @ AT[k]
  step2  M~2_T [128 (b,4f), 1024 n]  = sum_k h0[k].T @ A2T[k]
  step3  Phi_N [128 n, 8 kt, 512 (b,16j)] = |M~_T[:,k].T @ BDU4|  (ACT Abs)
  step4  APhi_T[c4][128 (b,16j), 1024 n] = sum_k Phi_N[k,c4].T @ AT[k]
  step5  Z2_T[c][128 (2b,64d), 1024 n] = BDW12[c].T @ M~2_T
                                       + BDCW2.T @ APhi_T[slice]   (+b2)
  step6  h2 = Prelu(Z2), mean-pool folded into ACT accum_out
  step7  y = Prelu(G_slices.T @ (Wp/1024) + bp)
"""

import numpy as np
import ml_dtypes

BF = ml_dtypes.bfloat16

B = 256
N = 1024
D1 = 64
D2 = 64
D_OUT = 128
NEG = 0.01
C1 = (1.0 + NEG) / 2.0
C2 = (1.0 - NEG) / 2.0
NCORES = 8
BS = B // NCORES          # 32 samples per core
NT = N // 128             # 8 node tiles
RB = 16                   # ridge basis size
FIT_REG = 1e-6


def _fit_ridge(W1, b1, rng_seed=12345, n_mc=120000, scales=None):
    """Least-squares fit |<m,w_d>+b1_d| ~= sum_j C[j,d] |<m,v_j>+v4_j| over
    isotropic gaussian m in R^3.  Uses only the weights (not x / topology).
    Returns V [4, RB], C [RB, 64]."""
    W1 = np.asarray(W1, np.float64)
    b1 = np.asarray(b1, np.float64)
    w4 = np.vstack([W1, b1[None, :]])                      # [4, 64]
    wn = w4 / (np.linalg.norm(w4, axis=0, keepdims=True) + 1e-30)
    rng = np.random.default_rng(rng_seed)
    idx = rng.choice(w4.shape[1], RB, replace=False)
    V = wn[:, idx].copy()
    for _ in range(300):                                   # projective k-means
        sim = np.abs(V.T @ wn)
        assign = sim.argmax(0)
        for j in range(RB):
            sel = wn[:, assign == j]
            if sel.shape[1] == 0:
                continue
            s = np.sign(V[:, j] @ sel)
            s[s == 0] = 1.0
            v = (sel * s).sum(1)
            nv = np.linalg.norm(v)
            if nv > 1e-9:
                V[:, j] = v / nv
    m = rng.standard_normal((n_mc, 3))
    if scales is not None:
        m *= scales[rng.integers(0, len(scales), n_mc)][:, None]
    m4 = np.concatenate([m, np.ones((n_mc, 1))], 1)
    Phi = np.abs(m4 @ V)
    T = np.abs(m4 @ w4)
    G = Phi.T @ Phi
    C = np.linalg.solve(G + FIT_REG * np.trace(G) / RB * np.eye(RB), Phi.T @ T)
    return V, C


def _build_host_constants(x, edge_index, W1, b1, W2, b2, Wp, bp):
    src = np.asarray(edge_index[0], dtype=np.int64)
    dst = np.asarray(edge_index[1], dtype=np.int64)
    deg = np.bincount(dst, minlength=N).astype(np.float64) + 1.0
    dinv = 1.0 / np.sqrt(deg)

    # AT[s, d] = A_hat[d, s]
    at = np.zeros((N, N), dtype=np.float64)
    np.add.at(at, (src, dst), dinv[src] * dinv[dst])
    at[np.arange(N), np.arange(N)] += dinv * dinv
    a2t = at @ at                                          # (A^2)[d,s]

    W1 = np.asarray(W1, np.float64)
    b1 = np.asarray(b1, np.float64)
    W2 = np.asarray(W2, np.float64)
    b2 = np.asarray(b2, np.float64)
    Wp = np.asarray(Wp, np.float64)
    bp = np.asarray(bp, np.float64)

    with_b1 = bool(np.any(b1))
    scales = np.sqrt((at ** 2).sum(0)) if with_b1 else None
    V, C = _fit_ridge(W1, b1, scales=scales)
    G1 = C1 * (np.vstack([W1, b1[None, :]]) @ W2)          # [4, 64]
    G2 = C2 * (C @ W2)                                     # [16, 64]

    # block-diagonal device matrices, bf16
    bdu4 = np.zeros((4 * BS, 16 * BS), dtype=np.float64)   # [128, 512]
    bdw12 = np.zeros((4 * BS, D2 * BS), dtype=np.float64)  # [128, 2048]
    for b in range(BS):
        bdu4[b * 4:(b + 1) * 4, b * 16:(b + 1) * 16] = V
        bdw12[b * 4:(b + 1) * 4, b * D2:(b + 1) * D2] = G1
    # BDCW2 x2: rows (2 samples x 16j) at partition offsets 0/32
    blk = np.zeros((32, 128), dtype=np.float64)
    blk[:16, :64] = G2
    blk[16:, 64:] = G2
    bdcw2 = np.tile(blk, (2, 1))                           # [64, 128]

    # x -> per-core h0 pack [128 node, NT, BS, 4] (f=3 zero)
    xr = np.asarray(x, np.float32).reshape(NCORES, BS, NT, 128, 3)
    h0 = np.zeros((NCORES, 128, NT, BS, 4), dtype=np.float32)
    h0[..., :3] = xr.transpose(0, 3, 2, 1, 4)

    consts = {
        "at": np.ascontiguousarray(at.astype(BF)),
        "a2t": np.ascontiguousarray(a2t.astype(BF)),
        "bdu4": bdu4.astype(BF),
        "bdw12": bdw12.astype(BF),
        "bdcw2": bdcw2.astype(BF),
        "wp_s": np.vstack([Wp / float(N), Wp / float(N)]).astype(np.float32),
        "b2col": np.tile(b2, 2)[None, :].astype(np.float32),   # [1, 128]
        "bprow": bp[None, :].astype(np.float32),               # [1, 128]
        "h0": h0.astype(BF),
        "with_b1": with_b1,
    }
    if with_b1:
        # ones / A*1 injected into the f=3 rows via outer-product matmuls
        sel = np.zeros((1, 128), dtype=np.float64)
        sel[0, 3::4] = 1.0
        consts["sel134"] = sel.astype(BF)
        consts["ones_n"] = np.ones((1, N), dtype=np.float64).astype(BF)
        consts["a1row"] = (at.T @ np.ones(N))[None, :].astype(BF)  # [1, N]
    return consts


_PROGRAM_CACHE = {}


def _build_program(with_b1, with_b2, with_bp):
    key = (with_b1, with_b2, with_bp)
    if key in _PROGRAM_CACHE:
        return _PROGRAM_CACHE[key]

    import concourse.mybir as mybir
    import concourse.tile as tile
    from concourse import bacc
    from contextlib import ExitStack

    f32 = mybir.dt.float32
    f32r = mybir.dt.float32r
    bf16 = mybir.dt.bfloat16
    AF = mybir.ActivationFunctionType

    nc = bacc.Bacc(trn_type="TRN2", target_bir_lowering=False, debug=False)

    h0_t = nc.dram_tensor("h0_s", [128, NT * BS * 4], bf16,
                          kind="ExternalInput").ap()
    at_t = nc.dram_tensor("at", [N, N], bf16, kind="ExternalInput").ap()
    a2t_t = nc.dram_tensor("a2t", [N, N], bf16, kind="ExternalInput").ap()
    bdu4_t = nc.dram_tensor("bdu4", [128, 16 * BS], bf16,
                            kind="ExternalInput").ap()
    bdw12_t = nc.dram_tensor("bdw12", [128, D2 * BS], bf16,
                             kind="ExternalInput").ap()
    bdcw2_t = nc.dram_tensor("bdcw2", [64, 128], bf16,
                             kind="ExternalInput").ap()
    wp_t = nc.dram_tensor("wp_s", [2 * D2, D_OUT], f32,
                          kind="ExternalInput").ap()
    b2_t = nc.dram_tensor("b2col", [1, 128], f32,
                          kind="ExternalInput").ap() if with_b2 else None
    bp_t = nc.dram_tensor("bprow", [1, D_OUT], f32,
                          kind="ExternalInput").ap() if with_bp else None
    if with_b1:
        sel_t = nc.dram_tensor("sel134", [1, 128], bf16,
                               kind="ExternalInput").ap()
        onesn_t = nc.dram_tensor("ones_n", [1, N], bf16,
                                 kind="ExternalInput").ap()
        a1_t = nc.dram_tensor("a1row", [1, N], bf16,
                              kind="ExternalInput").ap()
    y_t = nc.dram_tensor("y", [BS, D_OUT], f32, kind="ExternalOutput").ap()

    with tile.TileContext(nc) as tc, ExitStack() as es:
        const = es.enter_context(tc.tile_pool(name="const", bufs=1))
        work = es.enter_context(tc.tile_pool(name="work", bufs=1))

        h0_sb = const.tile([128, NT, BS, 4], bf16)
        at_sb = const.tile([128, NT, N], bf16)
        a2t_sb = const.tile([128, NT, N], bf16)
        bdu4_sb = const.tile([128, 16 * BS], bf16)
        bdw12_sb = const.tile([128, D2 * BS], bf16)
        bdcw2_sb = const.tile([64, 128], bf16)
        wp_sb = const.tile([2 * D2, D_OUT], f32r)
        wp_st = const.tile([2 * D2, D_OUT], f32)

        mt_sb = work.tile([128, N], bf16)       # M~_T
        m2t_sb = work.tile([128, N], bf16)      # M~2_T
        phi_sb = work.tile([128, NT, 16 * BS], bf16)
        g_sb = work.tile([128, BS // 2], f32)
        gr_sb = work.tile([128, BS // 2], f32r)
        ye_sb = work.tile([BS // 2, D_OUT], f32)
        yo_sb = work.tile([BS // 2, D_OUT], f32)

        # ---- loads: triggers spread over the three DGE-capable engines so
        # neither trigger-issue (~600ns each) nor per-queue transfer time
        # serializes ahead of compute.  sync: h0 + at[0..3]; scalar (idle
        # until step 3): at[4..7] + a2t[0..3]; gpsimd: a2t[4..7] + consts.
        h0_r = h0_t.rearrange("p (nt b f) -> p nt b f", nt=NT, b=BS, f=4)
        nc.sync.dma_start(out=h0_sb[:], in_=h0_r)
        at_r = at_t.rearrange("(kt p) d -> p kt d", kt=NT, p=128)
        for k in range(NT):
            eng = nc.sync if k < 4 else nc.scalar
            eng.dma_start(out=at_sb[:, k, :], in_=at_r[:, k, :])
        a2t_r = a2t_t.rearrange("(kt p) d -> p kt d", kt=NT, p=128)
        for k in range(NT):
            eng = nc.scalar if k < 4 else nc.gpsimd
            eng.dma_start(out=a2t_sb[:, k, :], in_=a2t_r[:, k, :])
        nc.gpsimd.dma_start(out=bdu4_sb[:], in_=bdu4_t)
        nc.gpsimd.dma_start(out=bdw12_sb[:], in_=bdw12_t)
        nc.gpsimd.dma_start(out=bdcw2_sb[:], in_=bdcw2_t)
        nc.gpsimd.dma_start(out=wp_st[:], in_=wp_t)
        nc.vector.tensor_copy(wp_sb[:], wp_st[:])
        # preload the Prelu activation table while the PE is busy: first real
        # Prelu otherwise eats a ~1.3us ACT_TABLE_LOAD mid-kernel
        warm_f = const.tile([1, 8], f32)
        warm_o = const.tile([1, 8], f32)
        nc.any.memset(warm_f[:], 0.0)
        nc.scalar.activation(warm_o[:], warm_f[:], AF.Prelu, alpha=NEG)
        if with_b1:
            sel_sb = const.tile([1, 128], bf16)
            onesn_sb = const.tile([1, N], bf16)
            a1_sb = const.tile([1, N], bf16)
            nc.gpsimd.dma_start(out=sel_sb[:], in_=sel_t)
            nc.gpsimd.dma_start(out=onesn_sb[:], in_=onesn_t)
            nc.gpsimd.dma_start(out=a1_sb[:], in_=a1_t)
        if with_b2 or with_bp:
            ones_f = const.tile([1, 512], f32)
            ones_sb = const.tile([1, 512], f32r)
            nc.any.memset(ones_f[:], 1.0)
            nc.vector.tensor_copy(ones_sb[:], ones_f[:])
        if with_b2:
            b2f_sb = const.tile([1, 128], f32)
            b2_sb = const.tile([1, 128], f32r)
            nc.gpsimd.dma_start(out=b2f_sb[:], in_=b2_t)
            nc.vector.tensor_copy(b2_sb[:], b2f_sb[:])
        if with_bp:
            bpf_sb = const.tile([1, D_OUT], f32)
            bp_sb = const.tile([1, D_OUT], f32r)
            nc.gpsimd.dma_start(out=bpf_sb[:], in_=bp_t)
            nc.vector.tensor_copy(bp_sb[:], bpf_sb[:])

        # ---- steps 1+2: M~_T / M~2_T = sum_k h0[k].T @ {AT,A2T}[k] ----
        with tc.tile_pool(name="ps_m", bufs=1, space="PSUM") as ps_m:
            m_ps = ps_m.tile([128, N], f32)
            m2_ps = ps_m.tile([128, N], f32)
            for k in range(NT):
                for n in range(2):
                    nc.tensor.matmul(
                        m_ps[:, n * 512:(n + 1) * 512],
                        h0_sb[:, k, :, :],
                        at_sb[:, k, n * 512:(n + 1) * 512],
                        start=(k == 0), stop=(k == NT - 1 and not with_b1),
                    )
            if with_b1:
                for n in range(2):
                    nc.tensor.matmul(   # += outer(sel134, ones): f=3 rows = 1
                        m_ps[:, n * 512:(n + 1) * 512], sel_sb[:],
                        onesn_sb[:, n * 512:(n + 1) * 512],
                        start=False, stop=True)
            for k in range(NT):
                for n in range(2):
                    nc.tensor.matmul(
                        m2_ps[:, n * 512:(n + 1) * 512],
                        h0_sb[:, k, :, :],
                        a2t_sb[:, k, n * 512:(n + 1) * 512],
                        start=(k == 0), stop=(k == NT - 1 and not with_b1),
                    )
            if with_b1:
                for n in range(2):
                    nc.tensor.matmul(   # += outer(sel134, A*ones)
                        m2_ps[:, n * 512:(n + 1) * 512], sel_sb[:],
                        a1_sb[:, n * 512:(n + 1) * 512],
                        start=False, stop=True)
            nc.vector.tensor_copy(mt_sb[:], m_ps[:])
            nc.vector.tensor_copy(m2t_sb[:], m2_ps[:])

        # ---- step 3: Phi_N[k] = |M~_T[:, k].T @ BDU4| ----
        with tc.tile_pool(name="ps_phi", bufs=2, space="PSUM") as ps_phi:
            for k in range(NT):
                p = ps_phi.tile([128, 16 * BS], f32, tag="phi")
                nc.tensor.matmul(p[:], mt_sb[:, k * 128:(k + 1) * 128],
                                 bdu4_sb[:], start=True, stop=True)
                nc.scalar.activation(phi_sb[:, k, :], p[:], AF.Abs)

        # ---- steps 4+5 interleaved ----
        with tc.tile_pool(name="ps_a", bufs=1, space="PSUM") as ps_a, \
             tc.tile_pool(name="ps_z", bufs=3, space="PSUM") as ps_z, \
             tc.tile_pool(name="sb_a", bufs=2) as sb_a, \
             tc.tile_pool(name="sb_h", bufs=2) as sb_h:
            aphi = []
            def emit_s4(c4):
                p4 = ps_a.tile([128, N], f32, tag="a4")
                for k in range(NT):
                    for n in range(2):
                        nc.tensor.matmul(
                            p4[:, n * 512:(n + 1) * 512],
                            phi_sb[:, k, c4 * 128:(c4 + 1) * 128],
                            at_sb[:, k, n * 512:(n + 1) * 512],
                            start=(k == 0), stop=(k == NT - 1),
                        )
                a_lo = sb_a.tile([64, N], bf16, tag="alo")
                a_hi = sb_a.tile([64, N], bf16, tag="ahi")
                nc.vector.tensor_copy(a_lo[:], p4[0:64, :])
                nc.vector.tensor_copy(a_hi[:], p4[64:128, :])
                aphi.append((a_lo, a_hi))

            def emit_s5(c4):
                for cc in range(4):
                    a_sb = aphi[c4][cc // 2]
                    off = (cc % 2) * 32
                    c = c4 * 4 + cc
                    p5 = ps_z.tile([128, N], f32, tag="z")
                    for n in range(2):
                        cs = slice(n * 512, (n + 1) * 512)
                        nc.tensor.matmul(
                            p5[:, cs], bdw12_sb[:, c * 128:(c + 1) * 128],
                            m2t_sb[:, cs], start=True, stop=False)
                        nc.tensor.matmul(
                            p5[:, cs], bdcw2_sb[off:off + 32, :],
                            a_sb[off:off + 32, cs],
                            start=False, stop=not with_b2)
                        if with_b2:
                            nc.tensor.matmul(
                                p5[:, cs], b2_sb[:], ones_sb[:, :512],
                                start=False, stop=True)
                    h2 = sb_h.tile([128, N], bf16, tag="h2")
                    nc.scalar.activation(h2[:], p5[:], AF.Prelu, alpha=NEG,
                                         accum_out=g_sb[:, c:c + 1])

            emit_s4(0)
            emit_s4(1)
            emit_s5(0)
            emit_s4(2)
            emit_s5(1)
            emit_s4(3)
            emit_s5(2)
            emit_s5(3)

        # ---- step 7: projection (same as baseline) ----
        nc.vector.tensor_copy(gr_sb[:], g_sb[:])
        with tc.tile_pool(name="ps_y", bufs=2, space="PSUM") as ps_y:
            for half, out_sb in ((0, ye_sb), (1, yo_sb)):
                y_ps = ps_y.tile([BS // 2, D_OUT], f32, tag="yps")
                nc.tensor.matmul(y_ps[:], gr_sb[half * D2:(half + 1) * D2, :],
                                 wp_sb[half * D2:(half + 1) * D2, :],
                                 start=True, stop=not with_bp)
                if with_bp:
                    nc.tensor.matmul(
                        y_ps[:], ones_sb[:, 0:BS // 2],
                        bp_sb[:], start=False, stop=True)
                nc.scalar.activation(out_sb[:], y_ps[:], AF.Prelu, alpha=NEG)

        y_r = y_t.rearrange("(c two) d -> two c d", two=2)
        nc.sync.dma_start(out=y_r[0, :, :], in_=ye_sb[:])
        nc.sync.dma_start(out=y_r[1, :, :], in_=yo_sb[:])

    nc.compile()
    _PROGRAM_CACHE[key] = nc
    return nc


def kernel(x, edge_index, W1, b1, W2, b2, Wp, bp, _trace=False):
    consts = _build_host_constants(x, edge_index, W1, b1, W2, b2, Wp, bp)
    with_b1 = consts["with_b1"]
    with_b2 = bool(np.any(consts["b2col"]))
    with_bp = bool(np.any(consts["bprow"]))

    nc = _build_program(with_b1, with_b2, with_bp)

    base = {"at": consts["at"], "a2t": consts["a2t"],
            "bdu4": consts["bdu4"], "bdw12": consts["bdw12"],
            "bdcw2": consts["bdcw2"], "wp_s": consts["wp_s"]}
    if with_b1:
        base["sel134"] = consts["sel134"]
        base["ones_n"] = consts["ones_n"]
        base["a1row"] = consts["a1row"]
    if with_b2:
        base["b2col"] = consts["b2col"]
    if with_bp:
        base["bprow"] = consts["bprow"]

    h0 = consts["h0"]
    in_maps = [dict(base, h0_s=np.ascontiguousarray(
        h0[c].reshape(128, NT * BS * 4))) for c in range(NCORES)]

    from concourse.bass_utils import run_bass_kernel_spmd
    res = run_bass_kernel_spmd(nc, in_maps, core_ids=list(range(NCORES)),
                               trace=_trace)
    y = np.concatenate([res.results[c]["y"] for c in range(NCORES)], axis=0)
    out = np.ascontiguousarray(y.astype(np.float32))
    if _trace:
        return out, res
    return out
